# revision 30
# baseline (speedup 1.0000x reference)
"""GQA attention kernel for 8 TRN2 NeuronCores.

Problem: B=2, T=2048, DIM=2048, 16 Q-heads, 4 KV-heads, head_dim=128,
causal mask, RoPE variant y = rot(x) * (sin + cos).

Sharding: core = b * 4 + g  (b in 0..1 batch, g in 0..3 kv-group).
Each core computes 4 Q-heads + its KV head for one batch element, plus the
row-sharded slice of the output projection; the host sums the 4 partials per
batch (the "all-reduce") and adds bout.

Device-side algorithm per core (all matmuls bf16 in / f32 accum):
  - RoPE's pair-swap/negate is folded into Wq/Wk rows on the host, so on
    device RoPE is an elementwise multiply by mT = (sin+cos).T * 128^-0.25
    (the extra 128^-0.25 on both q and k realizes the 1/sqrt(128) score
    scale).
  - Projections computed transposed: qT/kT/vT[d, t] = W.T-tile.T @ xT-chunk.
    Weight + x DMAs are interleaved with the first projection pass so the
    PE starts within a few us of kernel start.
  - v is transposed back to natural V[s, d] layout via DMA-transpose.
  - Scores computed TRANSPOSED: ST[s, t] = kT-tile.T @ qT-chunk, so after
    exp() the result is already P^T, which is exactly the lhsT the PV matmul
    needs -- no per-tile transposes of P.
  - No max-subtraction in softmax (scores are O(10); exp cannot overflow;
    masked entries are zeroed after exp, matching the reference's
    where(mask, s, -1e9) + softmax).
  - The attention phase is one flat software pipeline over (head, s-tile)
    units, crossing head boundaries: the score matmul of unit i is issued
    LAG=2 units ahead of its exp-dependent rowsum/PV matmuls, so the PE
    never drains on exp latency.  The previous t-chunk's out-projection
    groups are interleaved one-per-unit as PE filler work.
  - Row sums via ones-vector matmuls (M=1) into per-head [1, 512] PSUM
    accumulators; reciprocal via the single-instruction
    reciprocal_approx_fast; broadcast across partitions on GpSimd;
    normalization fused into the PSUM->SBUF eviction of the PV accumulator
    on the DVE.

Things measured SLOWER or broken on HW (do not re-add):
  - exp over [128, 1024] PSUM APs (crosses 2 banks) and/or DMA-transpose
    triggered from the Scalar HWDGE queue: hard device crash.
  - Weight DMAs on the Scalar HWDGE queue + staged [128, 2048] out-stores:
    uniform ~18%% matmul slowdown (SBUF/DGE interference), 299us -> 353us.
"""

import ml_dtypes
import numpy as np

import concourse.bacc as bacc
import concourse.bass as bass
import concourse.mybir as mybir
import concourse.tile as tile
from concourse import bass_utils

F32 = mybir.dt.float32
BF16 = mybir.dt.bfloat16

DIM = 2048
T = 2048
B = 2
HD = 128          # head dim
HPC = 4           # q heads per core
QW = HPC * HD     # 512 q rows per core
NC_T = 4          # t-chunks of 512
TC = 512          # t-chunk width
NT = T // 128     # 16 tiles of 128 along t/s/c


def build_kernel_nc():
    nc = bacc.Bacc("TRN2", target_bir_lowering=False, debug=False, num_devices=8)

    xT = nc.dram_tensor("xT", [DIM, T], BF16, kind="ExternalInput").ap()
    wqT = nc.dram_tensor("wqT", [DIM, QW], BF16, kind="ExternalInput").ap()
    wkT = nc.dram_tensor("wkT", [DIM, HD], BF16, kind="ExternalInput").ap()
    wvT = nc.dram_tensor("wvT", [DIM, HD], BF16, kind="ExternalInput").ap()
    woT = nc.dram_tensor("woT", [QW, DIM], BF16, kind="ExternalInput").ap()
    mT = nc.dram_tensor("mT", [HD, T], F32, kind="ExternalInput").ap()
    maskT = nc.dram_tensor("maskT", [128, 4 * TC], BF16, kind="ExternalInput").ap()
    out = nc.dram_tensor("out", [T, DIM], BF16, kind="ExternalOutput").ap()

    with tile.TileContext(nc) as tc:
        emit(tc, nc, xT, wqT, wkT, wvT, woT, mT, maskT, out)

    nc.compile()
    return nc


def emit(tc, nc, xT, wqT, wkT, wvT, woT, mT, maskT, out):
    from contextlib import ExitStack

    ctx = ExitStack()
    singles = ctx.enter_context(tc.tile_pool(name="singles", bufs=1))
    qkv = ctx.enter_context(tc.tile_pool(name="qkv", bufs=1))
    xs = ctx.enter_context(tc.tile_pool(name="xs", bufs=2))
    pts = ctx.enter_context(tc.tile_pool(name="pts", bufs=4))
    sm = ctx.enter_context(tc.tile_pool(name="sm", bufs=3))
    outs = ctx.enter_context(tc.tile_pool(name="outs", bufs=2))
    evs = ctx.enter_context(tc.tile_pool(name="evs", bufs=3))
    vtmp = ctx.enter_context(tc.tile_pool(name="vtmp", bufs=2))

    # ---- SBUF residents (DMAs for the big ones are emitted inside the
    # projection loop, interleaved with compute in consumption order) ----
    mT_sb = singles.tile([HD, T], F32, tag="mT")
    mask_sb = singles.tile([128, 4 * TC], BF16, tag="mask")
    wq_sb = singles.tile([128, NT, QW], BF16, tag="wq")
    wk_sb = singles.tile([128, NT, HD], BF16, tag="wk")
    wv_sb = singles.tile([128, NT, HD], BF16, tag="wv")
    wo_sb = singles.tile([128, HPC, DIM], BF16, tag="wo")

    wqr = wqT.rearrange("(a p) d -> p a d", p=128)
    wkr = wkT.rearrange("(a p) d -> p a d", p=128)
    wvr = wvT.rearrange("(a p) d -> p a d", p=128)
    wor = woT.rearrange("(a p) e -> p a e", p=128)
    xr = xT.rearrange("(a p) t -> p a t", p=128)

    ones_col = singles.tile([128, 1], BF16, tag="ones_col")
    nc.vector.memset(ones_col, 1.0)

    # ---- persistent activations ----
    qT_sb = qkv.tile([128, HPC, T], BF16, tag="qT")       # per head: [d, t]
    kT_sb = qkv.tile([128, T], BF16, tag="kT")            # [d, s]
    v_sb = qkv.tile([128, NT, HD], BF16, tag="v")         # per s-tile: [s, d]

    # =========== projections ===========
    with tc.tile_pool(name="proj_ps", bufs=6, space="PSUM") as pps:
        for tci in range(NC_T):
            tsl = bass.ts(tci, TC)
            xg = xs.tile([128, NT, TC], BF16, tag="x")
            if tci == 2:
                nc.sync.dma_start(out=mask_sb, in_=maskT)
            q_ps = [pps.tile([128, TC], F32, tag="ps", name=f"q_ps{h}")
                    for h in range(HPC)]
            k_ps = pps.tile([128, TC], F32, tag="ps")
            v_ps = pps.tile([128, TC], F32, tag="ps")
            for c in range(NT):
                if tci == 0:
                    # weight chunks interleaved with the x chunk they pair
                    # with, in consumption order: the first matmuls of a
                    # c-tile are the q ones, so wq + x go first and wk/wv
                    # trail them.
                    if c < 4:
                        nc.sync.dma_start(out=wq_sb[:, c:c + 1, :],
                                          in_=wqr[:, c:c + 1, :])
                        nc.sync.dma_start(out=xg[:, c:c + 1, :],
                                          in_=xr[:, c:c + 1, tsl])
                        nc.sync.dma_start(out=wk_sb[:, c:c + 1, :],
                                          in_=wkr[:, c:c + 1, :])
                        nc.sync.dma_start(out=wv_sb[:, c:c + 1, :],
                                          in_=wvr[:, c:c + 1, :])
                    elif c % 4 == 0:
                        nc.sync.dma_start(out=wq_sb[:, c:c + 4, :],
                                          in_=wqr[:, c:c + 4, :])
                        nc.sync.dma_start(out=xg[:, c:c + 4, :],
                                          in_=xr[:, c:c + 4, tsl])
                        if c == 4:
                            nc.sync.dma_start(out=wk_sb[:, 4:8, :],
                                              in_=wkr[:, 4:8, :])
                            nc.sync.dma_start(out=wv_sb[:, 4:8, :],
                                              in_=wvr[:, 4:8, :])
                        if c == 8:
                            nc.sync.dma_start(out=wk_sb[:, 8:, :],
                                              in_=wkr[:, 8:, :])
                            nc.sync.dma_start(out=wv_sb[:, 8:, :],
                                              in_=wvr[:, 8:, :])
                    if c == 11:
                        nc.sync.dma_start(out=mT_sb[:, 0:TC], in_=mT[:, 0:TC])
                else:
                    # x groups must stay ahead of the PE here; everything not
                    # needed until later (mT tail for the tci=1 eviction, wo
                    # for the attention-phase out-proj) is deferred so its
                    # transfers never back up the x prefetch.
                    if c % 4 == 0:
                        nc.sync.dma_start(out=xg[:, c:c + 4, :],
                                          in_=xr[:, c:c + 4, tsl])
                    if tci == 1 and c == 8:
                        nc.sync.dma_start(out=mT_sb[:, TC:], in_=mT[:, TC:])
                xch = xg[:, c, :]
                st, sp = (c == 0), (c == NT - 1)
                for h in range(HPC):
                    nc.tensor.matmul(q_ps[h], lhsT=wq_sb[:, c, h * HD:(h + 1) * HD],
                                     rhs=xch, start=st, stop=sp)
                    if sp:
                        nc.vector.tensor_mul(qT_sb[:, h, tsl], q_ps[h],
                                             mT_sb[:, tsl])
                nc.tensor.matmul(k_ps, lhsT=wk_sb[:, c, :], rhs=xch,
                                 start=st, stop=sp)
                if sp:
                    nc.vector.tensor_mul(kT_sb[:, tsl], k_ps, mT_sb[:, tsl])
                nc.tensor.matmul(v_ps, lhsT=wv_sb[:, c, :], rhs=xch,
                                 start=st, stop=sp)
                if sp:
                    vt = vtmp.tile([128, TC], BF16, tag="vt")
                    nc.scalar.copy(vt, v_ps)
                    for j in range(4):
                        si = 4 * tci + j
                        nc.sync.dma_start_transpose(
                            v_sb[:, si, :], vt[:, j * 128:(j + 1) * 128])

    # =========== attention + out-projection, per t-chunk ===========
    # Flat software pipeline over (h, si) units: the score matmul for unit i
    # runs LAG units ahead of its exp-dependent rowsum/PV matmuls, across
    # head boundaries, so the PE never drains on exp latency.  The previous
    # t-chunk's out-projection groups are interleaved as PE filler work.
    stp = ctx.enter_context(tc.tile_pool(name="st_ps", bufs=3, space="PSUM"))
    pvp = ctx.enter_context(tc.tile_pool(name="pv_ps", bufs=2, space="PSUM"))
    rsp = ctx.enter_context(tc.tile_pool(name="rs_ps", bufs=2, space="PSUM"))
    pop = ctx.enter_context(tc.tile_pool(name="po_ps", bufs=1, space="PSUM"))
    LAG = 2

    def normalize(h, pv, rs, outT_sb):
        recip = sm.tile([1, TC], F32, tag="recip")
        nc.vector.reciprocal_approx_fast(out=recip, in_=rs)
        bcb = sm.tile([128, TC], F32, tag="bc")
        nc.gpsimd.partition_broadcast(bcb, recip)
        nc.vector.tensor_mul(outT_sb[:, h, :], pv, bcb)

    def outproj_groups(tci, outT_sb, alt_bank=False):
        """Generator of emit-thunks, one per (tt, ec) out-proj group.
        alt_bank alternates the accumulator between two pools so that
        back-to-back groups do not serialize on one PSUM bank (only safe
        once all pv accumulators of the last t-chunk are normalized)."""
        for gi in range(16):
            tt, ec = gi // 4, gi % 4
            t0 = tci * TC + tt * 128

            def group(tt=tt, ec=ec, t0=t0, gi=gi):
                if alt_bank and gi % 2 == 1:
                    po = pvp.tile([128, TC], F32, tag="pv", name="po")
                else:
                    po = pop.tile([128, TC], F32, tag="po", name="po")
                for h in range(HPC):
                    nc.tensor.matmul(
                        po, lhsT=outT_sb[:, h, tt * 128:(tt + 1) * 128],
                        rhs=wo_sb[:, h, ec * TC:(ec + 1) * TC],
                        start=(h == 0), stop=(h == HPC - 1))
                ev = evs.tile([128, TC], BF16, tag="ev")
                if ec % 2 == 0:
                    nc.scalar.copy(ev, po)
                else:
                    nc.vector.tensor_copy(ev, po)
                # alt_bank (the final, non-interleaved out-proj) also
                # alternates the store between the two HWDGE rings so the
                # post-compute transfer drain halves; mid-kernel the Scalar
                # ring is busy feeding exp, so stores stay on Sync there.
                eng = nc.scalar if alt_bank and gi % 2 == 1 else nc.sync
                eng.dma_start(
                    out=out[t0:t0 + 128, ec * TC:(ec + 1) * TC], in_=ev)
            yield group

    prev_op = None  # pending out-proj generator of the previous t-chunk
    for tci in range(NC_T):
        nsi = 4 * tci + 4
        outT_sb = outs.tile([128, HPC, TC], BF16, tag="outT")
        heads = {}

        def unit_st(h, si):
            oi = si - 4 * tci
            lo = max(oi, 0) * 128
            t0 = tci * TC + lo
            st_t = stp.tile([128, TC], F32, tag="st")
            pt_t = pts.tile([128, TC], BF16, tag="pt")
            nc.tensor.matmul(st_t[:, lo:], lhsT=kT_sb[:, si * 128:(si + 1) * 128],
                             rhs=qT_sb[:, h, t0:t0 + TC - lo],
                             start=True, stop=True)
            return st_t, pt_t, lo, oi

        def unit_exp(st_t, pt_t, lo, oi):
            nc.scalar.activation(pt_t[:, lo:], st_t[:, lo:],
                                 mybir.ActivationFunctionType.Exp)
            if oi >= 0:
                nc.vector.tensor_mul(pt_t[:, lo:], pt_t[:, lo:],
                                     mask_sb[:, oi * TC + lo:(oi + 1) * TC])

        def unit_rspv(h, si, pt_t, lo):
            pv, rs = heads[h]
            first, last = (si == 0), (si == nsi - 1)
            nc.tensor.matmul(rs[:, lo:], lhsT=ones_col, rhs=pt_t[:, lo:],
                             start=first, stop=last)
            nc.tensor.matmul(pv[:, lo:], lhsT=v_sb[:, si, :], rhs=pt_t[:, lo:],
                             start=first, stop=last)
            if last:
                normalize(h, pv, rs, outT_sb)

        # out-proj filler: one group per unit, interleaved as PE filler work
        filler = list(outproj_groups(tci - 1, prev_op)) if prev_op is not None \
            else []

        pending = []
        for h in range(HPC):
            heads[h] = (pvp.tile([128, TC], F32, tag="pv", name=f"pv{h}"),
                        rsp.tile([1, TC], F32, tag="rs", name=f"rs{h}",
                                 padded_shape=[128, TC]))
            if tci == 0:
                # wo is first read by the out-proj fillers one t-chunk later;
                # its 2MB ride the idle Sync ring during attention tci=0.
                nc.sync.dma_start(out=wo_sb[:, h, :], in_=wor[:, h, :])
            for si in range(nsi):
                st_t, pt_t, lo, oi = unit_st(h, si)
                if len(pending) >= LAG:
                    unit_rspv(*pending.pop(0))
                if filler:
                    filler.pop(0)()
                unit_exp(st_t, pt_t, lo, oi)
                pending.append((h, si, pt_t, lo))
        while filler:
            filler.pop(0)()
        while pending:
            unit_rspv(*pending.pop(0))
        prev_op = outT_sb

    # last t-chunk's out-projection (nothing left to interleave with)
    for group in outproj_groups(NC_T - 1, prev_op, alt_bank=True):
        group()

    ctx.close()


# ---------------- host-side wrapper ----------------

_NC_CACHE = None


def _get_nc():
    global _NC_CACHE
    if _NC_CACHE is None:
        _NC_CACHE = build_kernel_nc()
    return _NC_CACHE


def _host_inputs(x, cos, sin, Wq, Wk, Wv, Wout):
    m = ((sin + cos) * np.float32(128.0 ** -0.25)).T  # [128, T]
    m = np.ascontiguousarray(m, dtype=np.float32)

    def rope_fold(W):
        Wr = np.empty_like(W)
        Wr[0::2] = -W[1::2]
        Wr[1::2] = W[0::2]
        return Wr

    Wq_r = rope_fold(np.asarray(Wq, dtype=np.float32))
    Wk_r = rope_fold(np.asarray(Wk, dtype=np.float32))

    # diagonal-band masks in ST layout: block oi: [s, t] valid iff t >= s + 128*oi
    s = np.arange(128)[:, None]
    t = np.arange(TC)[None, :]
    mask = np.concatenate(
        [np.where(t >= s + 128 * oi, 1.0, 0.0).astype(np.float32) for oi in range(4)],
        axis=1)
    mask = np.ascontiguousarray(mask).astype(ml_dtypes.bfloat16)

    maps = []
    for core in range(8):
        b, g = core // 4, core % 4
        maps.append({
            "xT": np.ascontiguousarray(x[b].T).astype(ml_dtypes.bfloat16),
            "wqT": np.ascontiguousarray(Wq_r[g * QW:(g + 1) * QW].T).astype(ml_dtypes.bfloat16),
            "wkT": np.ascontiguousarray(Wk_r[g * HD:(g + 1) * HD].T).astype(ml_dtypes.bfloat16),
            "wvT": np.ascontiguousarray(np.asarray(Wv, np.float32)[g * HD:(g + 1) * HD].T).astype(ml_dtypes.bfloat16),
            "woT": np.ascontiguousarray(np.asarray(Wout, np.float32)[:, g * QW:(g + 1) * QW].T).astype(ml_dtypes.bfloat16),
            "mT": m,
            "maskT": mask,
        })
    return maps


def kernel(x, cos, sin, mask, Wq, Wk, Wv, Wout, bout, _trace=False):
    nc = _get_nc()
    in_maps = _host_inputs(np.asarray(x, np.float32), np.asarray(cos, np.float32),
                           np.asarray(sin, np.float32), Wq, Wk, Wv, Wout)
    res = bass_utils.run_bass_kernel_spmd(nc, in_maps, core_ids=list(range(8)),
                                          trace=_trace)
    parts = [np.asarray(res.results[i]["out"]).astype(np.float32) for i in range(8)]
    bo = np.asarray(bout, np.float32)
    full = np.stack([parts[0] + parts[1] + parts[2] + parts[3] + bo,
                     parts[4] + parts[5] + parts[6] + parts[7] + bo])
    if _trace:
        return full.astype(np.float32), res
    return full.astype(np.float32)


# revision 34
# speedup vs baseline: 1.1683x; 1.1683x over previous
"""GQA attention kernel for 8 TRN2 NeuronCores.

Problem: B=2, T=2048, DIM=2048, 16 Q-heads, 4 KV-heads, head_dim=128,
causal mask, RoPE variant y = rot(x) * (sin + cos).

Sharding: core = b * 4 + g  (b in 0..1 batch, g in 0..3 kv-group).
Each core computes 4 Q-heads + its KV head for one batch element, plus the
row-sharded slice of the output projection; the host sums the 4 partials per
batch (the "all-reduce") and adds bout.

Device-side algorithm per core (all matmuls bf16 in / f32 accum):
  - RoPE's pair-swap/negate is folded into Wq/Wk rows on the host, so on
    device RoPE is an elementwise multiply by mT = (sin+cos).T * 128^-0.25
    (the extra 128^-0.25 on both q and k realizes the 1/sqrt(128) score
    scale).
  - Projections computed transposed: qT/kT/vT[d, t] = W.T-tile.T @ xT-chunk.
    Weight + x DMAs are interleaved with the first projection pass so the
    PE starts within a few us of kernel start.
  - v is transposed back to natural V[s, d] layout via DMA-transpose.
  - Scores computed TRANSPOSED: ST[s, t] = kT-tile.T @ qT-chunk, so after
    exp() the result is already P^T, which is exactly the lhsT the PV matmul
    needs -- no per-tile transposes of P.
  - No max-subtraction in softmax (scores are O(10); exp cannot overflow;
    masked entries are zeroed after exp, matching the reference's
    where(mask, s, -1e9) + softmax).
  - The attention phase is one flat software pipeline over (head, s-tile)
    units, crossing head boundaries: the score matmul of unit i is issued
    LAG=2 units ahead of its exp-dependent rowsum/PV matmuls, so the PE
    never drains on exp latency.  The previous t-chunk's out-projection
    groups are interleaved one-per-unit as PE filler work.
  - Row sums via ones-vector matmuls (M=1) into per-head [1, 512] PSUM
    accumulators; reciprocal via the single-instruction
    reciprocal_approx_fast; broadcast across partitions on GpSimd;
    normalization fused into the PSUM->SBUF eviction of the PV accumulator
    on the DVE.

Things measured SLOWER or broken on HW (do not re-add):
  - exp over [128, 1024] PSUM APs (crosses 2 banks) and/or DMA-transpose
    triggered from the Scalar HWDGE queue: hard device crash.
  - Weight DMAs on the Scalar HWDGE queue + staged [128, 2048] out-stores:
    uniform ~18%% matmul slowdown (SBUF/DGE interference), 299us -> 353us.
"""

import ml_dtypes
import numpy as np

import concourse.bacc as bacc
import concourse.bass as bass
import concourse.mybir as mybir
import concourse.tile as tile
from concourse import bass_utils

F32 = mybir.dt.float32
BF16 = mybir.dt.bfloat16

DIM = 2048
T = 2048
B = 2
HD = 128          # head dim
HPC = 4           # q heads per core
QW = HPC * HD     # 512 q rows per core
NC_T = 4          # t-chunks of 512
TC = 512          # t-chunk width
NT = T // 128     # 16 tiles of 128 along t/s/c


def build_kernel_nc():
    nc = bacc.Bacc("TRN2", target_bir_lowering=False, debug=False, num_devices=8)

    xT = nc.dram_tensor("xT", [DIM, T], BF16, kind="ExternalInput").ap()
    wqT = nc.dram_tensor("wqT", [DIM, QW], BF16, kind="ExternalInput").ap()
    wkT = nc.dram_tensor("wkT", [DIM, HD], BF16, kind="ExternalInput").ap()
    wvT = nc.dram_tensor("wvT", [DIM, HD], BF16, kind="ExternalInput").ap()
    woT = nc.dram_tensor("woT", [QW, DIM], BF16, kind="ExternalInput").ap()
    mT = nc.dram_tensor("mT", [HD, T], F32, kind="ExternalInput").ap()
    maskT = nc.dram_tensor("maskT", [128, 4 * TC], BF16, kind="ExternalInput").ap()
    # tile-major output: tile [ti*4 + ec] = rows ti*128..+128, cols ec*512..+512
    # of the logical [T, DIM] output. Contiguous 128KB per store DMA (the
    # row-major layout would write 1KB strided rows at ~half DMA efficiency);
    # the host un-tiles.
    out = nc.dram_tensor("out", [NT * 4, 128, TC], BF16,
                         kind="ExternalOutput").ap()

    with tile.TileContext(nc) as tc:
        emit(tc, nc, xT, wqT, wkT, wvT, woT, mT, maskT, out)

    nc.compile()
    return nc


def emit(tc, nc, xT, wqT, wkT, wvT, woT, mT, maskT, out):
    from contextlib import ExitStack

    ctx = ExitStack()
    singles = ctx.enter_context(tc.tile_pool(name="singles", bufs=1))
    qkv = ctx.enter_context(tc.tile_pool(name="qkv", bufs=1))
    xs = ctx.enter_context(tc.tile_pool(name="xs", bufs=2))
    pts = ctx.enter_context(tc.tile_pool(name="pts", bufs=4))
    sm = ctx.enter_context(tc.tile_pool(name="sm", bufs=3))
    outs = ctx.enter_context(tc.tile_pool(name="outs", bufs=2))
    evs = ctx.enter_context(tc.tile_pool(name="evs", bufs=3))
    vtmp = ctx.enter_context(tc.tile_pool(name="vtmp", bufs=2))

    # ---- SBUF residents (DMAs for the big ones are emitted inside the
    # projection loop, interleaved with compute in consumption order) ----
    mT_sb = singles.tile([HD, T], F32, tag="mT")
    mask_sb = singles.tile([128, 4 * TC], BF16, tag="mask")
    wq_sb = singles.tile([128, NT, QW], BF16, tag="wq")
    wk_sb = singles.tile([128, NT, HD], BF16, tag="wk")
    wv_sb = singles.tile([128, NT, HD], BF16, tag="wv")
    wo_sb = singles.tile([128, HPC, DIM], BF16, tag="wo")

    wqr = wqT.rearrange("(a p) d -> p a d", p=128)
    wkr = wkT.rearrange("(a p) d -> p a d", p=128)
    wvr = wvT.rearrange("(a p) d -> p a d", p=128)
    wor = woT.rearrange("(a p) e -> p a e", p=128)
    xr = xT.rearrange("(a p) t -> p a t", p=128)

    ones_col = singles.tile([128, 1], BF16, tag="ones_col")
    nc.vector.memset(ones_col, 1.0)

    # ---- persistent activations ----
    qT_sb = qkv.tile([128, HPC, T], BF16, tag="qT")       # per head: [d, t]
    kT_sb = qkv.tile([128, T], BF16, tag="kT")            # [d, s]
    v_sb = qkv.tile([128, NT, HD], BF16, tag="v")         # per s-tile: [s, d]

    # =========== projections ===========
    with tc.tile_pool(name="proj_ps", bufs=6, space="PSUM") as pps:
        for tci in range(NC_T):
            tsl = bass.ts(tci, TC)
            xg = xs.tile([128, NT, TC], BF16, tag="x")
            if tci == 2:
                nc.sync.dma_start(out=mask_sb, in_=maskT)
            q_ps = [pps.tile([128, TC], F32, tag="ps", name=f"q_ps{h}")
                    for h in range(HPC)]
            k_ps = pps.tile([128, TC], F32, tag="ps")
            v_ps = pps.tile([128, TC], F32, tag="ps")
            for c in range(NT):
                if tci == 0:
                    # weight chunks interleaved with the x chunk they pair
                    # with, in consumption order: the first matmuls of a
                    # c-tile are the q ones, so wq + x go first and wk/wv
                    # trail them.
                    if c < 4:
                        nc.sync.dma_start(out=wq_sb[:, c:c + 1, :],
                                          in_=wqr[:, c:c + 1, :])
                        nc.sync.dma_start(out=xg[:, c:c + 1, :],
                                          in_=xr[:, c:c + 1, tsl])
                        nc.sync.dma_start(out=wk_sb[:, c:c + 1, :],
                                          in_=wkr[:, c:c + 1, :])
                        nc.sync.dma_start(out=wv_sb[:, c:c + 1, :],
                                          in_=wvr[:, c:c + 1, :])
                    elif c % 4 == 0:
                        nc.sync.dma_start(out=wq_sb[:, c:c + 4, :],
                                          in_=wqr[:, c:c + 4, :])
                        nc.sync.dma_start(out=xg[:, c:c + 4, :],
                                          in_=xr[:, c:c + 4, tsl])
                        if c == 4:
                            nc.sync.dma_start(out=wk_sb[:, 4:8, :],
                                              in_=wkr[:, 4:8, :])
                            nc.sync.dma_start(out=wv_sb[:, 4:8, :],
                                              in_=wvr[:, 4:8, :])
                        if c == 8:
                            nc.sync.dma_start(out=wk_sb[:, 8:, :],
                                              in_=wkr[:, 8:, :])
                            nc.sync.dma_start(out=wv_sb[:, 8:, :],
                                              in_=wvr[:, 8:, :])
                    if c == 11:
                        nc.sync.dma_start(out=mT_sb[:, 0:TC], in_=mT[:, 0:TC])
                else:
                    # x groups must stay ahead of the PE here; everything not
                    # needed until later (mT tail for the tci=1 eviction, wo
                    # for the attention-phase out-proj) is deferred so its
                    # transfers never back up the x prefetch.
                    if c % 4 == 0:
                        nc.sync.dma_start(out=xg[:, c:c + 4, :],
                                          in_=xr[:, c:c + 4, tsl])
                    if tci == 1 and c == 8:
                        nc.sync.dma_start(out=mT_sb[:, TC:], in_=mT[:, TC:])
                xch = xg[:, c, :]
                st, sp = (c == 0), (c == NT - 1)
                for h in range(HPC):
                    nc.tensor.matmul(q_ps[h], lhsT=wq_sb[:, c, h * HD:(h + 1) * HD],
                                     rhs=xch, start=st, stop=sp)
                    if sp:
                        nc.vector.tensor_mul(qT_sb[:, h, tsl], q_ps[h],
                                             mT_sb[:, tsl])
                nc.tensor.matmul(k_ps, lhsT=wk_sb[:, c, :], rhs=xch,
                                 start=st, stop=sp)
                if sp:
                    nc.vector.tensor_mul(kT_sb[:, tsl], k_ps, mT_sb[:, tsl])
                nc.tensor.matmul(v_ps, lhsT=wv_sb[:, c, :], rhs=xch,
                                 start=st, stop=sp)
                if sp:
                    vt = vtmp.tile([128, TC], BF16, tag="vt")
                    nc.scalar.copy(vt, v_ps)
                    for j in range(4):
                        si = 4 * tci + j
                        nc.sync.dma_start_transpose(
                            v_sb[:, si, :], vt[:, j * 128:(j + 1) * 128])

    # =========== attention + out-projection, per t-chunk ===========
    # Flat software pipeline over (h, si) units: the score matmul for unit i
    # runs LAG units ahead of its exp-dependent rowsum/PV matmuls, across
    # head boundaries, so the PE never drains on exp latency.  The previous
    # t-chunk's out-projection groups are interleaved as PE filler work.
    stp = ctx.enter_context(tc.tile_pool(name="st_ps", bufs=3, space="PSUM"))
    pvp = ctx.enter_context(tc.tile_pool(name="pv_ps", bufs=2, space="PSUM"))
    rsp = ctx.enter_context(tc.tile_pool(name="rs_ps", bufs=2, space="PSUM"))
    pop = ctx.enter_context(tc.tile_pool(name="po_ps", bufs=1, space="PSUM"))
    LAG = 2

    def normalize(h, pv, rs, outT_sb):
        recip = sm.tile([1, TC], F32, tag="recip")
        nc.vector.reciprocal_approx_fast(out=recip, in_=rs)
        bcb = sm.tile([128, TC], F32, tag="bc")
        nc.gpsimd.partition_broadcast(bcb, recip)
        nc.vector.tensor_mul(outT_sb[:, h, :], pv, bcb)

    def outproj_groups(tci, outT_sb, alt_bank=False):
        """Generator of emit-thunks, one per (tt, ec) out-proj group.
        alt_bank alternates the accumulator between two pools so that
        back-to-back groups do not serialize on one PSUM bank (only safe
        once all pv accumulators of the last t-chunk are normalized)."""
        for gi in range(16):
            tt, ec = gi // 4, gi % 4
            ti = tci * 4 + tt

            def group(tt=tt, ec=ec, ti=ti, gi=gi):
                if alt_bank and gi % 2 == 1:
                    po = pvp.tile([128, TC], F32, tag="pv", name="po")
                else:
                    po = pop.tile([128, TC], F32, tag="po", name="po")
                for h in range(HPC):
                    nc.tensor.matmul(
                        po, lhsT=outT_sb[:, h, tt * 128:(tt + 1) * 128],
                        rhs=wo_sb[:, h, ec * TC:(ec + 1) * TC],
                        start=(h == 0), stop=(h == HPC - 1))
                ev = evs.tile([128, TC], BF16, tag="ev")
                if ec % 2 == 0:
                    nc.scalar.copy(ev, po)
                else:
                    nc.vector.tensor_copy(ev, po)
                # alt_bank (the final, non-interleaved out-proj) also
                # alternates the store between the two HWDGE rings so the
                # post-compute transfer drain halves; mid-kernel the Scalar
                # ring is busy feeding exp, so stores stay on Sync there.
                eng = nc.scalar if alt_bank and gi % 2 == 1 else nc.sync
                eng.dma_start(out=out[ti * 4 + ec], in_=ev)
            yield group

    prev_op = None  # pending out-proj generator of the previous t-chunk
    for tci in range(NC_T):
        nsi = 4 * tci + 4
        outT_sb = outs.tile([128, HPC, TC], BF16, tag="outT")
        heads = {}

        def unit_st(h, si):
            oi = si - 4 * tci
            lo = max(oi, 0) * 128
            t0 = tci * TC + lo
            st_t = stp.tile([128, TC], F32, tag="st")
            pt_t = pts.tile([128, TC], BF16, tag="pt")
            nc.tensor.matmul(st_t[:, lo:], lhsT=kT_sb[:, si * 128:(si + 1) * 128],
                             rhs=qT_sb[:, h, t0:t0 + TC - lo],
                             start=True, stop=True)
            return st_t, pt_t, lo, oi

        def unit_exp(st_t, pt_t, lo, oi):
            nc.scalar.activation(pt_t[:, lo:], st_t[:, lo:],
                                 mybir.ActivationFunctionType.Exp)
            if oi >= 0:
                nc.vector.tensor_mul(pt_t[:, lo:], pt_t[:, lo:],
                                     mask_sb[:, oi * TC + lo:(oi + 1) * TC])

        def unit_rspv(h, si, pt_t, lo):
            pv, rs = heads[h]
            first, last = (si == 0), (si == nsi - 1)
            nc.tensor.matmul(rs[:, lo:], lhsT=ones_col, rhs=pt_t[:, lo:],
                             start=first, stop=last)
            nc.tensor.matmul(pv[:, lo:], lhsT=v_sb[:, si, :], rhs=pt_t[:, lo:],
                             start=first, stop=last)
            if last:
                normalize(h, pv, rs, outT_sb)

        # out-proj filler: one group per unit, interleaved as PE filler work
        filler = list(outproj_groups(tci - 1, prev_op)) if prev_op is not None \
            else []

        pending = []
        for h in range(HPC):
            heads[h] = (pvp.tile([128, TC], F32, tag="pv", name=f"pv{h}"),
                        rsp.tile([1, TC], F32, tag="rs", name=f"rs{h}",
                                 padded_shape=[128, TC]))
            if tci == 0:
                # wo is first read by the out-proj fillers one t-chunk later;
                # its 2MB ride the idle Sync ring during attention tci=0.
                nc.sync.dma_start(out=wo_sb[:, h, :], in_=wor[:, h, :])
            for si in range(nsi):
                st_t, pt_t, lo, oi = unit_st(h, si)
                if len(pending) >= LAG:
                    unit_rspv(*pending.pop(0))
                if filler:
                    filler.pop(0)()
                unit_exp(st_t, pt_t, lo, oi)
                pending.append((h, si, pt_t, lo))
        while filler:
            filler.pop(0)()
        while pending:
            unit_rspv(*pending.pop(0))
        prev_op = outT_sb

    # last t-chunk's out-projection (nothing left to interleave with)
    for group in outproj_groups(NC_T - 1, prev_op, alt_bank=True):
        group()

    ctx.close()


# ---------------- host-side wrapper ----------------

_NC_CACHE = None


def _get_nc():
    global _NC_CACHE
    if _NC_CACHE is None:
        _NC_CACHE = build_kernel_nc()
    return _NC_CACHE


def _host_inputs(x, cos, sin, Wq, Wk, Wv, Wout):
    m = ((sin + cos) * np.float32(128.0 ** -0.25)).T  # [128, T]
    m = np.ascontiguousarray(m, dtype=np.float32)

    def rope_fold(W):
        Wr = np.empty_like(W)
        Wr[0::2] = -W[1::2]
        Wr[1::2] = W[0::2]
        return Wr

    Wq_r = rope_fold(np.asarray(Wq, dtype=np.float32))
    Wk_r = rope_fold(np.asarray(Wk, dtype=np.float32))

    # diagonal-band masks in ST layout: block oi: [s, t] valid iff t >= s + 128*oi
    s = np.arange(128)[:, None]
    t = np.arange(TC)[None, :]
    mask = np.concatenate(
        [np.where(t >= s + 128 * oi, 1.0, 0.0).astype(np.float32) for oi in range(4)],
        axis=1)
    mask = np.ascontiguousarray(mask).astype(ml_dtypes.bfloat16)

    maps = []
    for core in range(8):
        b, g = core // 4, core % 4
        maps.append({
            "xT": np.ascontiguousarray(x[b].T).astype(ml_dtypes.bfloat16),
            "wqT": np.ascontiguousarray(Wq_r[g * QW:(g + 1) * QW].T).astype(ml_dtypes.bfloat16),
            "wkT": np.ascontiguousarray(Wk_r[g * HD:(g + 1) * HD].T).astype(ml_dtypes.bfloat16),
            "wvT": np.ascontiguousarray(np.asarray(Wv, np.float32)[g * HD:(g + 1) * HD].T).astype(ml_dtypes.bfloat16),
            "woT": np.ascontiguousarray(np.asarray(Wout, np.float32)[:, g * QW:(g + 1) * QW].T).astype(ml_dtypes.bfloat16),
            "mT": m,
            "maskT": mask,
        })
    return maps


def kernel(x, cos, sin, mask, Wq, Wk, Wv, Wout, bout, _trace=False):
    nc = _get_nc()
    in_maps = _host_inputs(np.asarray(x, np.float32), np.asarray(cos, np.float32),
                           np.asarray(sin, np.float32), Wq, Wk, Wv, Wout)
    res = bass_utils.run_bass_kernel_spmd(nc, in_maps, core_ids=list(range(8)),
                                          trace=_trace)

    def untile(o):  # [64, 128, 512] tile-major -> [T, DIM]
        return (np.asarray(o).astype(np.float32)
                .reshape(NT, 4, 128, TC).transpose(0, 2, 1, 3)
                .reshape(T, DIM))

    parts = [untile(res.results[i]["out"]) for i in range(8)]
    bo = np.asarray(bout, np.float32)
    full = np.stack([parts[0] + parts[1] + parts[2] + parts[3] + bo,
                     parts[4] + parts[5] + parts[6] + parts[7] + bo])
    if _trace:
        return full.astype(np.float32), res
    return full.astype(np.float32)


# revision 36
# speedup vs baseline: 1.1876x; 1.0165x over previous
"""GQA attention kernel for 8 TRN2 NeuronCores.

Problem: B=2, T=2048, DIM=2048, 16 Q-heads, 4 KV-heads, head_dim=128,
causal mask, RoPE variant y = rot(x) * (sin + cos).

Sharding: core = b * 4 + g  (b in 0..1 batch, g in 0..3 kv-group).
Each core computes 4 Q-heads + its KV head for one batch element, plus the
row-sharded slice of the output projection; the host sums the 4 partials per
batch (the "all-reduce") and adds bout.

Device-side algorithm per core (all matmuls bf16 in / f32 accum):
  - RoPE's pair-swap/negate is folded into Wq/Wk rows on the host, so on
    device RoPE is an elementwise multiply by mT = (sin+cos).T * 128^-0.25
    (the extra 128^-0.25 on both q and k realizes the 1/sqrt(128) score
    scale).
  - Projections computed transposed: qT/kT/vT[d, t] = W.T-tile.T @ xT-chunk.
    Weight + x DMAs are interleaved with the first projection pass so the
    PE starts within a few us of kernel start.
  - v is transposed back to natural V[s, d] layout via DMA-transpose.
  - Scores computed TRANSPOSED: ST[s, t] = kT-tile.T @ qT-chunk, so after
    exp() the result is already P^T, which is exactly the lhsT the PV matmul
    needs -- no per-tile transposes of P.
  - No max-subtraction in softmax (scores are O(10); exp cannot overflow;
    masked entries are zeroed after exp, matching the reference's
    where(mask, s, -1e9) + softmax).
  - The attention phase is one flat software pipeline over (head, s-tile)
    units, crossing head boundaries: the score matmul of unit i is issued
    LAG=2 units ahead of its exp-dependent rowsum/PV matmuls, so the PE
    never drains on exp latency.  The previous t-chunk's out-projection
    groups are interleaved one-per-unit as PE filler work.
  - Row sums via ones-vector matmuls (M=1) into per-head [1, 512] PSUM
    accumulators; reciprocal via the single-instruction
    reciprocal_approx_fast; broadcast across partitions on GpSimd;
    normalization fused into the PSUM->SBUF eviction of the PV accumulator
    on the DVE.

Things measured SLOWER or broken on HW (do not re-add):
  - exp over [128, 1024] PSUM APs (crosses 2 banks) and/or DMA-transpose
    triggered from the Scalar HWDGE queue: hard device crash.
  - Weight DMAs on the Scalar HWDGE queue + staged [128, 2048] out-stores:
    uniform ~18%% matmul slowdown (SBUF/DGE interference), 299us -> 353us.
"""

import ml_dtypes
import numpy as np

import concourse.bacc as bacc
import concourse.bass as bass
import concourse.mybir as mybir
import concourse.tile as tile
from concourse import bass_utils

F32 = mybir.dt.float32
BF16 = mybir.dt.bfloat16

DIM = 2048
T = 2048
B = 2
HD = 128          # head dim
HPC = 4           # q heads per core
QW = HPC * HD     # 512 q rows per core
NC_T = 4          # t-chunks of 512
TC = 512          # t-chunk width
NT = T // 128     # 16 tiles of 128 along t/s/c


def build_kernel_nc():
    nc = bacc.Bacc("TRN2", target_bir_lowering=False, debug=False, num_devices=8)

    xT = nc.dram_tensor("xT", [DIM, T], BF16, kind="ExternalInput").ap()
    wqT = nc.dram_tensor("wqT", [DIM, QW], BF16, kind="ExternalInput").ap()
    wkT = nc.dram_tensor("wkT", [DIM, HD], BF16, kind="ExternalInput").ap()
    wvT = nc.dram_tensor("wvT", [DIM, HD], BF16, kind="ExternalInput").ap()
    woT = nc.dram_tensor("woT", [QW, DIM], BF16, kind="ExternalInput").ap()
    mT = nc.dram_tensor("mT", [HD, T], F32, kind="ExternalInput").ap()
    maskT = nc.dram_tensor("maskT", [128, 4 * TC], BF16, kind="ExternalInput").ap()
    # tile-major output: tile [ti*4 + ec] = rows ti*128..+128, cols ec*512..+512
    # of the logical [T, DIM] output. Contiguous 128KB per store DMA (the
    # row-major layout would write 1KB strided rows at ~half DMA efficiency);
    # the host un-tiles.
    out = nc.dram_tensor("out", [NT * 4, 128, TC], BF16,
                         kind="ExternalOutput").ap()

    with tile.TileContext(nc) as tc:
        emit(tc, nc, xT, wqT, wkT, wvT, woT, mT, maskT, out)

    nc.compile()
    return nc


def emit(tc, nc, xT, wqT, wkT, wvT, woT, mT, maskT, out):
    from contextlib import ExitStack

    ctx = ExitStack()
    singles = ctx.enter_context(tc.tile_pool(name="singles", bufs=1))
    qkv = ctx.enter_context(tc.tile_pool(name="qkv", bufs=1))
    xs = ctx.enter_context(tc.tile_pool(name="xs", bufs=2))
    pts = ctx.enter_context(tc.tile_pool(name="pts", bufs=4))
    sm = ctx.enter_context(tc.tile_pool(name="sm", bufs=3))
    outs = ctx.enter_context(tc.tile_pool(name="outs", bufs=2))
    evs = ctx.enter_context(tc.tile_pool(name="evs", bufs=3))
    vtmp = ctx.enter_context(tc.tile_pool(name="vtmp", bufs=2))

    # ---- SBUF residents (DMAs for the big ones are emitted inside the
    # projection loop, interleaved with compute in consumption order) ----
    mT_sb = singles.tile([HD, T], F32, tag="mT")
    mask_sb = singles.tile([128, 4 * TC], BF16, tag="mask")
    wq_sb = singles.tile([128, NT, QW], BF16, tag="wq")
    wk_sb = singles.tile([128, NT, HD], BF16, tag="wk")
    wv_sb = singles.tile([128, NT, HD], BF16, tag="wv")
    wo_sb = singles.tile([128, HPC, DIM], BF16, tag="wo")

    wqr = wqT.rearrange("(a p) d -> p a d", p=128)
    wkr = wkT.rearrange("(a p) d -> p a d", p=128)
    wvr = wvT.rearrange("(a p) d -> p a d", p=128)
    wor = woT.rearrange("(a p) e -> p a e", p=128)
    xr = xT.rearrange("(a p) t -> p a t", p=128)

    ones_col = singles.tile([128, 1], BF16, tag="ones_col")
    nc.vector.memset(ones_col, 1.0)

    # ---- persistent activations ----
    qT_sb = qkv.tile([128, HPC, T], BF16, tag="qT")       # per head: [d, t]
    kT_sb = qkv.tile([128, T], BF16, tag="kT")            # [d, s]
    v_sb = qkv.tile([128, NT, HD], BF16, tag="v")         # per s-tile: [s, d]

    # =========== projections ===========
    with tc.tile_pool(name="proj_ps", bufs=6, space="PSUM") as pps:
        for tci in range(NC_T):
            tsl = bass.ts(tci, TC)
            xg = xs.tile([128, NT, TC], BF16, tag="x")
            if tci == 2:
                nc.sync.dma_start(out=mask_sb, in_=maskT)
            q_ps = [pps.tile([128, TC], F32, tag="ps", name=f"q_ps{h}")
                    for h in range(HPC)]
            k_ps = pps.tile([128, TC], F32, tag="ps")
            v_ps = pps.tile([128, TC], F32, tag="ps")
            for c in range(NT):
                if tci == 0:
                    # weight chunks interleaved with the x chunk they pair
                    # with, in consumption order: the first matmuls of a
                    # c-tile are the q ones, so wq + x go first and wk/wv
                    # trail them.
                    if c < 4:
                        nc.sync.dma_start(out=wq_sb[:, c:c + 1, :],
                                          in_=wqr[:, c:c + 1, :])
                        nc.sync.dma_start(out=xg[:, c:c + 1, :],
                                          in_=xr[:, c:c + 1, tsl])
                        nc.sync.dma_start(out=wk_sb[:, c:c + 1, :],
                                          in_=wkr[:, c:c + 1, :])
                        nc.sync.dma_start(out=wv_sb[:, c:c + 1, :],
                                          in_=wvr[:, c:c + 1, :])
                    elif c % 4 == 0:
                        nc.sync.dma_start(out=wq_sb[:, c:c + 4, :],
                                          in_=wqr[:, c:c + 4, :])
                        nc.sync.dma_start(out=xg[:, c:c + 4, :],
                                          in_=xr[:, c:c + 4, tsl])
                        if c == 4:
                            nc.sync.dma_start(out=wk_sb[:, 4:8, :],
                                              in_=wkr[:, 4:8, :])
                            nc.sync.dma_start(out=wv_sb[:, 4:8, :],
                                              in_=wvr[:, 4:8, :])
                        if c == 8:
                            nc.sync.dma_start(out=wk_sb[:, 8:, :],
                                              in_=wkr[:, 8:, :])
                            nc.sync.dma_start(out=wv_sb[:, 8:, :],
                                              in_=wvr[:, 8:, :])
                    if c == 11:
                        nc.sync.dma_start(out=mT_sb[:, 0:TC], in_=mT[:, 0:TC])
                else:
                    # x groups must stay ahead of the PE here; everything not
                    # needed until later (mT tail for the tci=1 eviction, wo
                    # for the attention-phase out-proj) is deferred so its
                    # transfers never back up the x prefetch.
                    if c % 4 == 0:
                        nc.sync.dma_start(out=xg[:, c:c + 4, :],
                                          in_=xr[:, c:c + 4, tsl])
                    if tci == 1 and c == 8:
                        nc.sync.dma_start(out=mT_sb[:, TC:], in_=mT[:, TC:])
                xch = xg[:, c, :]
                st, sp = (c == 0), (c == NT - 1)
                for h in range(HPC):
                    nc.tensor.matmul(q_ps[h], lhsT=wq_sb[:, c, h * HD:(h + 1) * HD],
                                     rhs=xch, start=st, stop=sp)
                    if sp:
                        nc.vector.tensor_mul(qT_sb[:, h, tsl], q_ps[h],
                                             mT_sb[:, tsl])
                nc.tensor.matmul(k_ps, lhsT=wk_sb[:, c, :], rhs=xch,
                                 start=st, stop=sp)
                if sp:
                    nc.vector.tensor_mul(kT_sb[:, tsl], k_ps, mT_sb[:, tsl])
                nc.tensor.matmul(v_ps, lhsT=wv_sb[:, c, :], rhs=xch,
                                 start=st, stop=sp)
                if sp:
                    vt = vtmp.tile([128, TC], BF16, tag="vt")
                    nc.scalar.copy(vt, v_ps)
                    for j in range(4):
                        si = 4 * tci + j
                        nc.sync.dma_start_transpose(
                            v_sb[:, si, :], vt[:, j * 128:(j + 1) * 128])

    # =========== attention + out-projection, per t-chunk ===========
    # Flat software pipeline over (h, si) units: the score matmul for unit i
    # runs LAG units ahead of its exp-dependent rowsum/PV matmuls, across
    # head boundaries, so the PE never drains on exp latency.  The previous
    # t-chunk's out-projection groups are interleaved as PE filler work.
    stp = ctx.enter_context(tc.tile_pool(name="st_ps", bufs=3, space="PSUM"))
    pvp = ctx.enter_context(tc.tile_pool(name="pv_ps", bufs=2, space="PSUM"))
    rsp = ctx.enter_context(tc.tile_pool(name="rs_ps", bufs=2, space="PSUM"))
    pop = ctx.enter_context(tc.tile_pool(name="po_ps", bufs=1, space="PSUM"))
    LAG = 2

    def normalize(h, pv, rs, outT_sb):
        recip = sm.tile([1, TC], F32, tag="recip")
        nc.vector.reciprocal_approx_fast(out=recip, in_=rs)
        bcb = sm.tile([128, TC], F32, tag="bc")
        nc.gpsimd.partition_broadcast(bcb, recip)
        nc.vector.tensor_mul(outT_sb[:, h, :], pv, bcb)

    def outproj_groups(tci, outT_sb, alt_bank=False):
        """Generator of emit-thunks, one per (tt, ec) out-proj group.
        alt_bank alternates the accumulator between two pools so that
        back-to-back groups do not serialize on one PSUM bank (only safe
        once all pv accumulators of the last t-chunk are normalized)."""
        for gi in range(16):
            tt, ec = gi // 4, gi % 4
            ti = tci * 4 + tt

            def group(tt=tt, ec=ec, ti=ti, gi=gi):
                if alt_bank and gi % 2 == 1:
                    po = pvp.tile([128, TC], F32, tag="pv", name="po")
                else:
                    po = pop.tile([128, TC], F32, tag="po", name="po")
                for h in range(HPC):
                    nc.tensor.matmul(
                        po, lhsT=outT_sb[:, h, tt * 128:(tt + 1) * 128],
                        rhs=wo_sb[:, h, ec * TC:(ec + 1) * TC],
                        start=(h == 0), stop=(h == HPC - 1))
                ev = evs.tile([128, TC], BF16, tag="ev")
                if ec % 2 == 0:
                    nc.scalar.copy(ev, po)
                else:
                    nc.vector.tensor_copy(ev, po)
                # alt_bank (the final, non-interleaved out-proj) also
                # alternates the store between the two HWDGE rings so the
                # post-compute transfer drain halves; mid-kernel the Scalar
                # ring is busy feeding exp, so stores stay on Sync there.
                eng = nc.scalar if alt_bank and gi % 2 == 1 else nc.sync
                eng.dma_start(out=out[ti * 4 + ec], in_=ev)
            yield group

    prev_op = None  # pending out-proj generator of the previous t-chunk
    for tci in range(NC_T):
        nsi = 4 * tci + 4
        outT_sb = outs.tile([128, HPC, TC], BF16, tag="outT")
        heads = {}

        def unit_st(h, si):
            oi = si - 4 * tci
            lo = max(oi, 0) * 128
            t0 = tci * TC + lo
            st_t = stp.tile([128, TC], F32, tag="st")
            pt_t = pts.tile([128, TC], BF16, tag="pt")
            nc.tensor.matmul(st_t[:, lo:], lhsT=kT_sb[:, si * 128:(si + 1) * 128],
                             rhs=qT_sb[:, h, t0:t0 + TC - lo],
                             start=True, stop=True)
            return st_t, pt_t, lo, oi

        def unit_exp(st_t, pt_t, lo, oi):
            nc.scalar.activation(pt_t[:, lo:], st_t[:, lo:],
                                 mybir.ActivationFunctionType.Exp)
            if oi >= 0:
                nc.vector.tensor_mul(pt_t[:, lo:], pt_t[:, lo:],
                                     mask_sb[:, oi * TC + lo:(oi + 1) * TC])

        def unit_rspv(h, si, pt_t, lo):
            pv, rs = heads[h]
            first, last = (si == 0), (si == nsi - 1)
            nc.tensor.matmul(rs[:, lo:], lhsT=ones_col, rhs=pt_t[:, lo:],
                             start=first, stop=last)
            nc.tensor.matmul(pv[:, lo:], lhsT=v_sb[:, si, :], rhs=pt_t[:, lo:],
                             start=first, stop=last)
            if last:
                normalize(h, pv, rs, outT_sb)

        # out-proj filler: one group per unit, interleaved as PE filler work
        filler = list(outproj_groups(tci - 1, prev_op)) if prev_op is not None \
            else []

        pending = []
        for h in range(HPC):
            heads[h] = (pvp.tile([128, TC], F32, tag="pv", name=f"pv{h}"),
                        rsp.tile([1, TC], F32, tag="rs", name=f"rs{h}",
                                 padded_shape=[128, TC]))
            if tci == 0:
                # wo is first read by the out-proj fillers one t-chunk later;
                # its 2MB ride the idle Sync ring during attention tci=0.
                nc.sync.dma_start(out=wo_sb[:, h, :], in_=wor[:, h, :])
            for si in range(nsi):
                st_t, pt_t, lo, oi = unit_st(h, si)
                if len(pending) >= LAG:
                    unit_rspv(*pending.pop(0))
                if filler:
                    filler.pop(0)()
                unit_exp(st_t, pt_t, lo, oi)
                pending.append((h, si, pt_t, lo))
        while filler:
            filler.pop(0)()
        while pending:
            unit_rspv(*pending.pop(0))
        prev_op = outT_sb

    # last t-chunk's out-projection (nothing left to interleave with)
    for group in outproj_groups(NC_T - 1, prev_op, alt_bank=True):
        group()

    ctx.close()


# ---------------- host-side wrapper ----------------

_NC_CACHE = None


def _get_nc():
    global _NC_CACHE
    if _NC_CACHE is None:
        _NC_CACHE = build_kernel_nc()
    return _NC_CACHE


def _host_inputs(x, cos, sin, Wq, Wk, Wv, Wout):
    m = ((sin + cos) * np.float32(128.0 ** -0.25)).T  # [128, T]
    m = np.ascontiguousarray(m, dtype=np.float32)

    def rope_fold(W):
        Wr = np.empty_like(W)
        Wr[0::2] = -W[1::2]
        Wr[1::2] = W[0::2]
        return Wr

    Wq_r = rope_fold(np.asarray(Wq, dtype=np.float32))
    Wk_r = rope_fold(np.asarray(Wk, dtype=np.float32))

    # diagonal-band masks in ST layout: block oi: [s, t] valid iff t >= s + 128*oi
    s = np.arange(128)[:, None]
    t = np.arange(TC)[None, :]
    mask = np.concatenate(
        [np.where(t >= s + 128 * oi, 1.0, 0.0).astype(np.float32) for oi in range(4)],
        axis=1)
    mask = np.ascontiguousarray(mask).astype(ml_dtypes.bfloat16)

    maps = []
    for core in range(8):
        b, g = core // 4, core % 4
        maps.append({
            "xT": np.ascontiguousarray(x[b].T).astype(ml_dtypes.bfloat16),
            "wqT": np.ascontiguousarray(Wq_r[g * QW:(g + 1) * QW].T).astype(ml_dtypes.bfloat16),
            "wkT": np.ascontiguousarray(Wk_r[g * HD:(g + 1) * HD].T).astype(ml_dtypes.bfloat16),
            "wvT": np.ascontiguousarray(np.asarray(Wv, np.float32)[g * HD:(g + 1) * HD].T).astype(ml_dtypes.bfloat16),
            "woT": np.ascontiguousarray(np.asarray(Wout, np.float32)[:, g * QW:(g + 1) * QW].T).astype(ml_dtypes.bfloat16),
            "mT": m,
            "maskT": mask,
        })
    return maps


def kernel(x, cos, sin, mask, Wq, Wk, Wv, Wout, bout, _trace=False):
    nc = _get_nc()
    in_maps = _host_inputs(np.asarray(x, np.float32), np.asarray(cos, np.float32),
                           np.asarray(sin, np.float32), Wq, Wk, Wv, Wout)
    res = bass_utils.run_bass_kernel_spmd(nc, in_maps, core_ids=list(range(8)),
                                          trace=_trace)

    def untile(o):  # [64, 128, 512] tile-major -> [T, DIM]
        return (np.asarray(o).astype(np.float32)
                .reshape(NT, 4, 128, TC).transpose(0, 2, 1, 3)
                .reshape(T, DIM))

    parts = [untile(res.results[i]["out"]) for i in range(8)]
    bo = np.asarray(bout, np.float32)
    full = np.stack([parts[0] + parts[1] + parts[2] + parts[3] + bo,
                     parts[4] + parts[5] + parts[6] + parts[7] + bo])
    if _trace:
        return full.astype(np.float32), res
    return full.astype(np.float32)
